# revision 1
# baseline (speedup 1.0000x reference)
"""Trainium2 Bass kernel for nn_DecoderLayer (causal linear self-attn +
linear cross-attn + FFN decoder layer), SPMD over 8 NeuronCores.

Sharding: tokens are split 8 ways — core c handles batch c//4, token
quarter c%4 (512 tokens). All weights are replicated. The two
sequence-global reductions (causal-attention prefix state and
cross-attention context sums) are tiny (133 KB); since cross-core
collectives are not available in this runtime, the kernel runs as two
NEFF launches with a host-side exchange of those tensors in between.

Precision: projections/FFN run on the PE in float32r (full-rate fp32),
attention internals in bf16 with fp32 PSUM accumulation.
"""

import sys

sys.path.insert(0, "/opt/trn_rl_repo")

import numpy as np

import concourse.bass as bass
import concourse.mybir as mybir
import concourse.tile as tile
from concourse import bacc, bass_utils
from concourse.masks import make_identity, make_upper_triangular

F32 = mybir.dt.float32
F32R = mybir.dt.float32r
BF16 = mybir.dt.bfloat16
AF = mybir.ActivationFunctionType
ALU = mybir.AluOpType
AX = mybir.AxisListType

P = 128
B, N, DIM, H, FF = 2, 2048, 512, 8, 2048
E = DIM // H  # 64
T = N // 4  # 512 tokens per core
NT = T // P  # 4 token tiles per core
KF = DIM // P  # 4 feature tiles
NFF = FF // P  # 16
LN_EPS = 1e-5
ATTN_EPS = 1e-6
QS_SCALE = float(E) ** -0.5  # 0.125
N_CORES = 8
GROUPS = [[0, 1, 2, 3], [4, 5, 6, 7]]


class Evict:
    """Round-robin PSUM->SBUF eviction across DVE and ACT to balance load."""

    def __init__(self, nc):
        self.nc = nc
        self.i = 0

    def copy(self, out, in_):
        if self.i % 2 == 0:
            self.nc.vector.tensor_copy(out, in_)
        else:
            self.nc.scalar.copy(out, in_)
        self.i += 1

    def add(self, out, in0, in1):
        self.nc.vector.tensor_add(out, in0, in1)
        self.i += 1


def _layernorm(nc, sbuf, eps_tile, x3, g_bc, b_bc, out3, trivial):
    """LayerNorm over the feature (free) axis for (P, NT, DIM) tiles.

    trivial=True skips the affine (g==1, b==0).
    """
    for mt in range(NT):
        x = x3[:, mt, :]
        s = sbuf.tile([P, 1], F32, name="ln_s")
        nc.vector.reduce_sum(s[:], x, axis=AX.X)
        negmu = sbuf.tile([P, 1], F32, name="ln_negmu")
        nc.scalar.mul(negmu[:], s[:], -1.0 / DIM)
        sq = sbuf.tile([P, DIM], F32, name="ln_sq")
        ssq = sbuf.tile([P, 1], F32, name="ln_ssq")
        nc.scalar.activation(sq[:], x, AF.Square, bias=negmu[:],
                             accum_out=ssq[:])
        std = sbuf.tile([P, 1], F32, name="ln_std")
        nc.scalar.activation(std[:], ssq[:], AF.Sqrt, bias=eps_tile[:],
                             scale=1.0 / DIM)
        rstd = sbuf.tile([P, 1], F32, name="ln_rstd")
        nc.vector.reciprocal(rstd[:], std[:])
        if trivial:
            nc.vector.tensor_scalar(out3[:, mt, :], x, negmu[:], rstd[:],
                                    ALU.add, ALU.mult)
        else:
            xh = sbuf.tile([P, DIM], F32, name="ln_xh")
            nc.vector.tensor_scalar(xh[:], x, negmu[:], rstd[:],
                                    ALU.add, ALU.mult)
            xg = sbuf.tile([P, DIM], F32, name="ln_xg")
            nc.vector.tensor_mul(xg[:], xh[:], g_bc[:])
            nc.vector.tensor_add(out3[:, mt, :], xg[:], b_bc[:])


def _softmax_heads_bf16(nc, sbuf, src3, out_bf3):
    """qs = softmax(q, per 64-wide head) * E**-0.5 -> bf16.

    No max-subtraction: |q| is O(1) here, exp is safe in fp32.
    """
    for mt in range(NT):
        qe = sbuf.tile([P, H, E], F32, name="sm_qe")
        nc.scalar.activation(
            qe[:], src3[:, mt, :].rearrange("p (h e) -> p h e", e=E), AF.Exp
        )
        qsum = sbuf.tile([P, H], F32, name="sm_qsum")
        nc.vector.reduce_sum(qsum[:], qe[:], axis=AX.X)
        qrec = sbuf.tile([P, H], F32, name="sm_qrec")
        nc.vector.reciprocal(qrec[:], qsum[:])
        nc.vector.tensor_scalar_mul(qrec[:], qrec[:], QS_SCALE)
        nc.gpsimd.tensor_mul(
            out_bf3[:, mt, :].rearrange("p (h e) -> p h e", e=E),
            qe[:],
            qrec[:, :, None].to_broadcast((P, H, E)),
        )


def _build_v1(nc, pool, v_bf3, name):
    """(P, NT, H, E+1) bf16: per chunk/head [v | 1] (ones-column trick).

    Runs on GpSimd (SBUF-only) to keep DVE free.
    """
    v1 = pool.tile([P, NT, H, E + 1], BF16, name=name)
    nc.gpsimd.memset(v1[:], 1.0)
    for mt in range(NT):
        nc.gpsimd.tensor_copy(
            v1[:, mt, :, 0:E],
            v_bf3[:, mt, :].rearrange("p (h e) -> p h e", e=E),
        )
    return v1


def _dma_bcast(nc, pool, dram_ap, width, name):
    t = pool.tile([P, width], F32, name=name)
    nc.sync.dma_start(t[:], dram_ap[None, :].to_broadcast((P, width)))
    return t


def build_m1(trivial):
    """Module 1: qvk & kv projections; emits qs/ke/v (bf16), per-chunk
    self-attn sums S and the cross-attn context partial sums."""
    nc = bacc.Bacc(None, target_bir_lowering=False, debug=False,
                   num_devices=N_CORES)
    x_d = nc.dram_tensor("x", [T, DIM], F32, kind="ExternalInput")
    mem_d = nc.dram_tensor("mem", [T, DIM], F32, kind="ExternalInput")
    wqvk_d = nc.dram_tensor("W_qvk", [DIM, 3 * DIM], F32R, kind="ExternalInput")
    wkv_d = nc.dram_tensor("W_kv", [DIM, 2 * DIM], F32R, kind="ExternalInput")
    if not trivial:
        bqvk_d = nc.dram_tensor("b_qvk", [3 * DIM], F32, kind="ExternalInput")
        bkv_d = nc.dram_tensor("b_kv", [2 * DIM], F32, kind="ExternalInput")

    qsT_o = nc.dram_tensor("qsT_o", [P, KF * NT * P], BF16, kind="ExternalOutput")
    poi_o = nc.dram_tensor("poi_o", [T, H * (E + 1)], F32, kind="ExternalOutput")
    s_o = nc.dram_tensor("s_o", [NT, H * E, E + 1], F32, kind="ExternalOutput")
    ctx_o = nc.dram_tensor("ctx_o", [H * E, E + 1], F32, kind="ExternalOutput")

    with tile.TileContext(nc) as tc:
        with (
            tc.tile_pool(name="const", bufs=1) as cpool,
            tc.tile_pool(name="acts", bufs=1) as acts,
            tc.tile_pool(name="w", bufs=1) as wpool,
            tc.tile_pool(name="sb", bufs=3) as sbuf,
            tc.tile_pool(name="pt", bufs=2, space="PSUM") as pp_t,
            tc.tile_pool(name="pb", bufs=3, space="PSUM") as pp_b,
            tc.tile_pool(name="ps", bufs=3, space="PSUM") as pp_s,
        ):
            ev = Evict(nc)
            ident = cpool.tile([P, P], F32, name="ident")
            make_identity(nc, ident[:])
            ident_bf = cpool.tile([P, P], BF16, name="ident_bf")
            make_identity(nc, ident_bf[:])
            if not trivial:
                bqvk_bc = _dma_bcast(nc, cpool, bqvk_d.ap(), 3 * DIM, "bqvk_bc")
                bkv_bc = _dma_bcast(nc, cpool, bkv_d.ap(), 2 * DIM, "bkv_bc")

            xn = acts.tile([P, NT, DIM], F32, name="xn")
            nc.sync.dma_start(xn[:], x_d.ap().rearrange("(m p) n -> p m n", p=P))
            wqvk = wpool.tile([P, KF, 3 * DIM], F32R, name="wqvk")
            nc.sync.dma_start(wqvk[:], wqvk_d.ap().rearrange("(k p) n -> p k n", p=P))
            memn = acts.tile([P, NT, DIM], F32, name="memn")
            nc.sync.dma_start(memn[:], mem_d.ap().rearrange("(m p) n -> p m n", p=P))
            wkv = wpool.tile([P, KF, 2 * DIM], F32R, name="wkv")
            nc.sync.dma_start(wkv[:], wkv_d.ap().rearrange("(k p) n -> p k n", p=P))

            xT = acts.tile([P, KF, T], F32R, name="xT")
            memT = acts.tile([P, KF, T], F32R, name="memT")
            for kf in range(KF):
                for mt in range(NT):
                    pt = pp_t.tile([P, P], F32, name="tpsum", tag="t128")
                    nc.tensor.transpose(pt[:], xn[:, mt, kf * P:(kf + 1) * P],
                                        ident[:])
                    ev.copy(xT[:, kf, mt * P:(mt + 1) * P], pt[:])
                    pt2 = pp_t.tile([P, P], F32, name="tpsum", tag="t128")
                    nc.tensor.transpose(pt2[:], memn[:, mt, kf * P:(kf + 1) * P],
                                        ident[:])
                    ev.copy(memT[:, kf, mt * P:(mt + 1) * P], pt2[:])

            qvk_n = acts.tile([P, NT, 3 * DIM], F32, name="qvk_n")
            for mt in range(NT):
                for nb in range(3):
                    ps = pp_b.tile([P, 512], F32, name="proj_ps", tag="proj")
                    for kf in range(KF):
                        nc.tensor.matmul(
                            ps[:], xT[:, kf, mt * P:(mt + 1) * P],
                            wqvk[:, kf, nb * 512:(nb + 1) * 512],
                            start=(kf == 0), stop=(kf == KF - 1),
                        )
                    dst = qvk_n[:, mt, nb * 512:(nb + 1) * 512]
                    if trivial:
                        ev.copy(dst, ps[:])
                    else:
                        ev.add(dst, ps[:], bqvk_bc[:, nb * 512:(nb + 1) * 512])

            kv_n = acts.tile([P, NT, 2 * DIM], F32, name="kv_n")
            for mt in range(NT):
                for nb in range(2):
                    ps = pp_b.tile([P, 512], F32, name="proj_ps", tag="proj")
                    for kf in range(KF):
                        nc.tensor.matmul(
                            ps[:], memT[:, kf, mt * P:(mt + 1) * P],
                            wkv[:, kf, nb * 512:(nb + 1) * 512],
                            start=(kf == 0), stop=(kf == KF - 1),
                        )
                    dst = kv_n[:, mt, nb * 512:(nb + 1) * 512]
                    if trivial:
                        ev.copy(dst, ps[:])
                    else:
                        ev.add(dst, ps[:], bkv_bc[:, nb * 512:(nb + 1) * 512])

            qs_bf = acts.tile([P, NT, DIM], BF16, name="qs_bf")
            _softmax_heads_bf16(nc, sbuf, qvk_n[:, :, 0:DIM], qs_bf)
            ke_bf = acts.tile([P, NT, DIM], BF16, name="ke_bf")
            v_bf = acts.tile([P, NT, DIM], BF16, name="v_bf")
            for mt in range(NT):
                nc.scalar.activation(ke_bf[:, mt, :],
                                     qvk_n[:, mt, 2 * DIM:3 * DIM], AF.Exp)
                nc.gpsimd.tensor_copy(v_bf[:, mt, :], qvk_n[:, mt, DIM:2 * DIM])
            ones_bf = cpool.tile([P, 1], BF16, name="ones_bf")
            nc.gpsimd.memset(ones_bf[:], 1.0)

            # transposed qs / ke; qsT also shipped to M2 (single tile/DMA)
            qsT = acts.tile([P, KF, NT, P], BF16, name="qsT")
            keT = acts.tile([P, KF, NT, P], BF16, name="keT")
            for hp in range(KF):
                for mt in range(NT):
                    pt = pp_t.tile([P, P], BF16, name="tp_bf", tag="t128")
                    nc.tensor.transpose(pt[:], qs_bf[:, mt, hp * P:(hp + 1) * P],
                                        ident_bf[:])
                    ev.copy(qsT[:, hp, mt, :], pt[:])
                    pt2 = pp_t.tile([P, P], BF16, name="tp_bf", tag="t128")
                    nc.tensor.transpose(pt2[:], ke_bf[:, mt, hp * P:(hp + 1) * P],
                                        ident_bf[:])
                    ev.copy(keT[:, hp, mt, :], pt2[:])
            nc.sync.dma_start(
                qsT_o.ap(), qsT[:].rearrange("p k m n -> p (k m n)")
            )

            # intra-chunk causal attention: poi = maskedA^T.T @ [v|1],
            # 4 heads batched per PSUM tile, one eviction per group
            umask = cpool.tile([P, P], BF16, name="umask")
            make_upper_triangular(nc, umask[:], val=1.0, diag=True)
            poi_sb = acts.tile([P, NT, H, E + 1], F32, name="poi_sb")
            poi_v = poi_o.ap().rearrange("(m p) (h n) -> p m h n", p=P, n=E + 1)
            for j in range(NT):
                for g in range(2):
                    po4 = pp_s.tile([P, 4 * (E + 1)], F32, name="poi_ps",
                                    tag="small")
                    for i in range(4):
                        h = 4 * g + i
                        hp, prow = h // 2, (h % 2) * E
                        qsT_h = qsT[prow:prow + E, hp, j, :]
                        keT_h = keT[prow:prow + E, hp, j, :]
                        pa = pp_b.tile([P, P], F32, name="at_ps", tag="proj")
                        nc.tensor.matmul(pa[:], keT_h, qsT_h,
                                         start=True, stop=True)
                        amt = sbuf.tile([P, P], BF16, name="amt")
                        nc.vector.tensor_mul(amt[:], pa[:], umask[:])
                        sl = po4[:, i * (E + 1):(i + 1) * (E + 1)]
                        nc.tensor.matmul(sl[:, 0:E], amt[:],
                                         v_bf[:, j, h * E:(h + 1) * E],
                                         start=True, stop=True)
                        nc.tensor.matmul(sl[:, E:E + 1], amt[:], ones_bf[:],
                                         start=True, stop=True)
                    ev.copy(poi_sb[:, j, 4 * g:4 * g + 4, :],
                            po4[:].rearrange("p (h n) -> p h n", n=E + 1))
                nc.sync.dma_start(poi_v[:, j], poi_sb[:, j])
            s_sb = acts.tile([P, NT, KF, E + 1], F32, name="s_sb")
            s_v = s_o.ap().rearrange("j (s p) n -> p j s n", p=P)
            for j in range(NT):
                for hp in range(KF):
                    ps = pp_s.tile([P, E + 1], F32, name="s_ps", tag="small")
                    for i in range(2):
                        h = 2 * hp + i
                        sl = ps[i * E:(i + 1) * E, :]
                        nc.tensor.matmul(sl[:, 0:E],
                                         ke_bf[:, j, h * E:(h + 1) * E],
                                         v_bf[:, j, h * E:(h + 1) * E],
                                         start=True, stop=True)
                        nc.tensor.matmul(sl[:, E:E + 1],
                                         ke_bf[:, j, h * E:(h + 1) * E],
                                         ones_bf[:], start=True, stop=True)
                    ev.copy(s_sb[:, j, hp, :], ps[:])
                nc.sync.dma_start(s_v[:, j], s_sb[:, j])

            kec_bf = acts.tile([P, NT, DIM], BF16, name="kec_bf")
            vc_bf = acts.tile([P, NT, DIM], BF16, name="vc_bf")
            for mt in range(NT):
                nc.scalar.activation(kec_bf[:, mt, :], kv_n[:, mt, 0:DIM], AF.Exp)
                nc.gpsimd.tensor_copy(vc_bf[:, mt, :], kv_n[:, mt, DIM:2 * DIM])
            ctx_sb = acts.tile([P, KF, E + 1], F32, name="ctx_sb")
            for hp in range(KF):
                ps = pp_s.tile([P, E + 1], F32, name="ctx_ps", tag="small")
                for i in range(2):
                    h = 2 * hp + i
                    sl = ps[i * E:(i + 1) * E, :]
                    for j in range(NT):
                        nc.tensor.matmul(sl[:, 0:E],
                                         kec_bf[:, j, h * E:(h + 1) * E],
                                         vc_bf[:, j, h * E:(h + 1) * E],
                                         start=(j == 0), stop=(j == NT - 1))
                        nc.tensor.matmul(sl[:, E:E + 1],
                                         kec_bf[:, j, h * E:(h + 1) * E],
                                         ones_bf[:],
                                         start=(j == 0), stop=(j == NT - 1))
                ev.copy(ctx_sb[:, hp, :], ps[:])
            nc.sync.dma_start(ctx_o.ap().rearrange("(s p) n -> p s n", p=P),
                              ctx_sb[:])

    nc.compile()
    return nc


def build_m2(trivial, use_pool=True):
    """Module 2: self-attention, LN1, cross-attention, LN2, FFN, LN3."""
    nc = bacc.Bacc(None, target_bir_lowering=False, debug=False,
                   num_devices=N_CORES)
    x_d = nc.dram_tensor("x", [T, DIM], F32, kind="ExternalInput")
    qsT_d = nc.dram_tensor("qsT_i", [P, KF * NT * P], BF16, kind="ExternalInput")
    poi_d = nc.dram_tensor("poi_i", [T, H * (E + 1)], F32, kind="ExternalInput")
    pcore_d = nc.dram_tensor("p_core", [H * E, E + 1], F32, kind="ExternalInput")
    s_d = nc.dram_tensor("s_i", [NT, H * E, E + 1], F32, kind="ExternalInput")
    ctx_d = nc.dram_tensor("ctx", [H * E, E + 1], F32, kind="ExternalInput")
    wq_d = nc.dram_tensor("W_q", [DIM, DIM], F32R, kind="ExternalInput")
    wff1_d = nc.dram_tensor("W_ff1", [DIM, FF], F32R, kind="ExternalInput")
    wff2_d = nc.dram_tensor("W_ff2", [FF, DIM], F32R, kind="ExternalInput")
    bff1_d = nc.dram_tensor("b_ff1", [FF], F32, kind="ExternalInput")
    lng = {}
    if not trivial:
        bq_d = nc.dram_tensor("b_q", [DIM], F32, kind="ExternalInput")
        bff2_d = nc.dram_tensor("b_ff2", [DIM], F32, kind="ExternalInput")
        for i in (1, 2, 3):
            lng[f"g{i}"] = nc.dram_tensor(f"ln{i}_g", [DIM], F32,
                                          kind="ExternalInput")
            lng[f"b{i}"] = nc.dram_tensor(f"ln{i}_b", [DIM], F32,
                                          kind="ExternalInput")
    out_d = nc.dram_tensor("out", [T, DIM], F32, kind="ExternalOutput")

    with tile.TileContext(nc) as tc:
        with (
            tc.tile_pool(name="const", bufs=1) as cpool,
            tc.tile_pool(name="acts", bufs=1) as acts,
            tc.tile_pool(name="w", bufs=1) as wpool,
            tc.tile_pool(name="sb", bufs=3) as sbuf,
            tc.tile_pool(name="pw1", bufs=2) as pw1,
            tc.tile_pool(name="ph1", bufs=3) as ph1,
            tc.tile_pool(name="pw2", bufs=2) as pw2,
            tc.tile_pool(name="pt", bufs=2, space="PSUM") as pp_t,
            tc.tile_pool(name="pb", bufs=4, space="PSUM") as pp_b,
            tc.tile_pool(name="ps", bufs=2, space="PSUM") as pp_s,
        ):
            ev = Evict(nc)
            ident = cpool.tile([P, P], F32, name="ident")
            make_identity(nc, ident[:])
            ident_bf = cpool.tile([P, P], BF16, name="ident_bf")
            make_identity(nc, ident_bf[:])
            eps_t = cpool.tile([P, 1], F32, name="eps_t")
            nc.vector.memset(eps_t[:], LN_EPS)
            g_bc = {k: None for k in ("g1", "b1", "g2", "b2", "g3", "b3")}
            if not trivial:
                bq_bc = _dma_bcast(nc, cpool, bq_d.ap(), DIM, "bq_bc")
                bff2_bc = _dma_bcast(nc, cpool, bff2_d.ap(), DIM, "bff2_bc")
                for i in (1, 2, 3):
                    g_bc[f"g{i}"] = _dma_bcast(nc, cpool, lng[f"g{i}"].ap(),
                                               DIM, f"g{i}bc")
                    g_bc[f"b{i}"] = _dma_bcast(nc, cpool, lng[f"b{i}"].ap(),
                                               DIM, f"b{i}bc")
            bff1T = cpool.tile([P, NFF], F32, name="bff1T")
            nc.sync.dma_start(bff1T[:], bff1_d.ap().rearrange("(m p) -> p m", p=P))

            # loads ordered by first use: attention inputs first, x last
            pcore = acts.tile([P, KF, E + 1], F32, name="pcore")
            nc.sync.dma_start(pcore[:], pcore_d.ap().rearrange("(s p) n -> p s n", p=P))
            qsT = acts.tile([P, KF, NT, P], BF16, name="qsT")
            nc.sync.dma_start(qsT[:].rearrange("p k m n -> p (k m n)"),
                              qsT_d.ap())
            poi_v = poi_d.ap().rearrange("(m p) (h n) -> p m h n", p=P, n=E + 1)
            poi_sb = acts.tile([P, NT, H, E + 1], F32, name="poi_sb")
            for j in range(NT):
                nc.sync.dma_start(poi_sb[:, j], poi_v[:, j])
            s_in = acts.tile([P, NT, KF, E + 1], F32, name="s_in")
            nc.sync.dma_start(s_in[:], s_d.ap().rearrange("j (s p) n -> p j s n", p=P))
            ctx_in = acts.tile([P, KF, E + 1], F32, name="ctx_in")
            nc.sync.dma_start(ctx_in[:], ctx_d.ap().rearrange("(s p) n -> p s n", p=P))
            xn = acts.tile([P, NT, DIM], F32, name="xn")
            nc.sync.dma_start(xn[:], x_d.ap().rearrange("(m p) n -> p m n", p=P))

            # causal linear self-attention: inter-chunk part + combine with
            # the intra-chunk numerators (poi) from module 1. The prefix
            # state P1 is kept batched for all 8 heads as a (P, KF, E+1)
            # tile; epilogues run 4 heads at a time.
            attn_n = acts.tile([P, NT, DIM], F32, name="attn_n")
            s_in_bf = acts.tile([P, NT, KF, E + 1], BF16, name="s_in_bf")
            nc.vector.tensor_copy(s_in_bf[:], s_in[:])
            peng = nc.gpsimd if use_pool else nc.vector
            p1 = sbuf.tile([P, KF, E + 1], BF16, name="p1", tag="p1")
            peng.tensor_copy(p1[:], pcore[:])
            for j in range(NT):
                for g in range(2):
                    comb = sbuf.tile([P, 4, E + 1], F32, name="compo")
                    for i in range(4):
                        h = 4 * g + i
                        hp, prow = h // 2, (h % 2) * E
                        po = pp_s.tile([P, E + 1], F32, name="o_ps",
                                       tag="small")
                        nc.tensor.matmul(
                            po[:],
                            qsT[prow:prow + E, hp, j, :],
                            p1[prow:prow + E, hp, :],
                            start=True, stop=True,
                        )
                        nc.vector.tensor_add(comb[:, i, :], po[:],
                                             poi_sb[:, j, 4 * g + i, :])
                    den = sbuf.tile([P, 4], F32, name="den")
                    nc.scalar.activation(den[:], comb[:, :, E], AF.Copy,
                                         bias=ATTN_EPS * QS_SCALE)
                    dinv = sbuf.tile([P, 4], F32, name="dinv")
                    nc.vector.reciprocal(dinv[:], den[:])
                    nc.vector.tensor_mul(
                        attn_n[:, j, :].rearrange(
                            "p (h e) -> p h e", e=E)[:, 4 * g:4 * g + 4, :],
                        comb[:, :, 0:E],
                        dinv[:, :, None].to_broadcast((P, 4, E)),
                    )
                if j < NT - 1:
                    p1n = sbuf.tile([P, KF, E + 1], BF16, name="p1", tag="p1")
                    peng.tensor_add(p1n[:], p1[:], s_in_bf[:, j])
                    p1 = p1n

            # residual + LN1
            res1 = acts.tile([P, NT, DIM], F32, name="res1")
            for mt in range(NT):
                peng.tensor_add(res1[:, mt, :], attn_n[:, mt, :],
                                xn[:, mt, :])
            ln1_n = acts.tile([P, NT, DIM], F32, name="ln1_n")
            _layernorm(nc, sbuf, eps_t, res1, g_bc["g1"], g_bc["b1"], ln1_n,
                       trivial)
            ln1T = acts.tile([P, KF, T], F32R, name="ln1T")
            for kf in range(KF):
                for mt in range(NT):
                    pt = pp_t.tile([P, P], F32, name="tpsum", tag="t128")
                    nc.tensor.transpose(pt[:], ln1_n[:, mt, kf * P:(kf + 1) * P],
                                        ident[:])
                    ev.copy(ln1T[:, kf, mt * P:(mt + 1) * P], pt[:])

            # cross-attention: q projection + softmax
            wq = wpool.tile([P, KF, DIM], F32R, name="wq")
            nc.sync.dma_start(wq[:], wq_d.ap().rearrange("(k p) n -> p k n", p=P))
            qc_n = acts.tile([P, NT, DIM], F32, name="qc_n", tag="xn")
            for mt in range(NT):
                ps = pp_b.tile([P, 512], F32, name="proj_ps", tag="proj")
                for kf in range(KF):
                    nc.tensor.matmul(ps[:], ln1T[:, kf, mt * P:(mt + 1) * P],
                                     wq[:, kf, :],
                                     start=(kf == 0), stop=(kf == KF - 1))
                if trivial:
                    ev.copy(qc_n[:, mt, :], ps[:])
                else:
                    ev.add(qc_n[:, mt, :], ps[:], bq_bc[:])
            qsc_bf = acts.tile([P, NT, DIM], BF16, name="qsc_bf", tag="qs_bf")
            _softmax_heads_bf16(nc, sbuf, qc_n, qsc_bf)
            qscT = []
            for hp in range(KF):
                qscT.append(acts.tile([P, NT, P], BF16, name=f"qscT{hp}",
                                      tag=f"qsT{hp}"))
                for mt in range(NT):
                    pt = pp_t.tile([P, P], BF16, name="tp_bf", tag="t128")
                    nc.tensor.transpose(pt[:], qsc_bf[:, mt, hp * P:(hp + 1) * P],
                                        ident_bf[:])
                    ev.copy(qscT[hp][:, mt, :], pt[:])

            # normalize context: ctx[:, :E] / ctx[:, E]
            ctxn = acts.tile([P, KF, E], BF16, name="ctxn")
            crec = sbuf.tile([P, KF], F32, name="crec")
            nc.vector.reciprocal(crec[:], ctx_in[:, :, E])
            for s in range(KF):
                nc.vector.tensor_scalar_mul(ctxn[:, s, :], ctx_in[:, s, 0:E],
                                            crec[:, s:s + 1])

            # cross attention output + residual(ln1), 4-head batches
            cr_n = acts.tile([P, NT, DIM], F32, name="cr_n", tag="attn_n")
            for mt in range(NT):
                for h in range(H):
                    hp, prow = h // 2, (h % 2) * E
                    po = pp_s.tile([P, E], F32, name="co_ps", tag="small")
                    nc.tensor.matmul(po[:],
                                     qscT[hp][prow:prow + E, mt, :],
                                     ctxn[prow:prow + E, hp, :],
                                     start=True, stop=True)
                    ev.add(cr_n[:, mt, h * E:(h + 1) * E], po[:],
                           ln1_n[:, mt, h * E:(h + 1) * E])

            ln2_n = acts.tile([P, NT, DIM], F32, name="ln2_n")
            _layernorm(nc, sbuf, eps_t, cr_n, g_bc["g2"], g_bc["b2"], ln2_n,
                       trivial)
            ln2T = acts.tile([P, KF, T], F32R, name="ln2T", tag="ln1T")
            for kf in range(KF):
                for mt in range(NT):
                    pt = pp_t.tile([P, P], F32, name="tpsum", tag="t128")
                    nc.tensor.transpose(pt[:], ln2_n[:, mt, kf * P:(kf + 1) * P],
                                        ident[:])
                    ev.copy(ln2T[:, kf, mt * P:(mt + 1) * P], pt[:])

            # FFN — stream W_ff1/W_ff2 in 1MB chunks (4 ff blocks each);
            # h1 per 128-wide ff block feeds 4 concurrently-held y PSUM
            # accumulators (one per token tile).
            wff1_v = wff1_d.ap().rearrange("(k p) n -> p k n", p=P)
            wff2_v = wff2_d.ap().rearrange("(k p) n -> p k n", p=P)
            yps = [pp_b.tile([P, 512], F32, name=f"y_ps{mt}", tag="proj")
                   for mt in range(NT)]
            CH = 4  # ff blocks per streamed chunk
            for kc in range(NFF // CH):
                w1t = pw1.tile([P, KF, CH * P], F32R, name="w1s", tag="w1s")
                nc.sync.dma_start(
                    w1t[:], wff1_v[:, :, kc * CH * P:(kc + 1) * CH * P])
                w2t = pw2.tile([P, CH, 512], F32R, name="w2s", tag="w2s")
                nc.sync.dma_start(w2t[:], wff2_v[:, kc * CH:(kc + 1) * CH, :])
                for ki in range(CH):
                    kff = kc * CH + ki
                    h1ps = pp_t.tile([P, 512], F32, name="h1_ps", tag="t128")
                    for kf in range(KF):
                        nc.tensor.matmul(
                            h1ps[:], w1t[:, kf, ki * P:(ki + 1) * P],
                            ln2T[:, kf, :],
                            start=(kf == 0), stop=(kf == KF - 1))
                    # relu(h1 + bias) fused on DVE (idle during FFN)
                    h1t = ph1.tile([P, 512], F32R, name="h1s", tag="h1s")
                    nc.vector.tensor_scalar(h1t[:], h1ps[:],
                                            bff1T[:, kff:kff + 1], 0.0,
                                            ALU.add, ALU.max)
                    for mt in range(NT):
                        nc.tensor.matmul(yps[mt][:],
                                         h1t[:, mt * P:(mt + 1) * P],
                                         w2t[:, ki, :],
                                         start=(kff == 0),
                                         stop=(kff == NFF - 1))

            y_n = acts.tile([P, NT, DIM], F32, name="y_n", tag="res1")
            for mt in range(NT):
                if trivial:
                    nc.vector.tensor_add(y_n[:, mt, :], yps[mt][:],
                                         ln2_n[:, mt, :])
                else:
                    t = sbuf.tile([P, DIM], F32, name="ffn_t")
                    nc.vector.tensor_add(t[:], yps[mt][:], ln2_n[:, mt, :])
                    nc.vector.tensor_add(y_n[:, mt, :], t[:], bff2_bc[:])

            out_n = acts.tile([P, NT, DIM], F32, name="out_n", tag="xn")
            _layernorm(nc, sbuf, eps_t, y_n, g_bc["g3"], g_bc["b3"], out_n,
                       trivial)
            nc.sync.dma_start(out_d.ap().rearrange("(m p) n -> p m n", p=P),
                              out_n[:])

    nc.compile()
    return nc


_CACHE = {}
LAST_EXEC_NS = []


def _modules(trivial):
    key = ("m", trivial)
    if key not in _CACHE:
        _CACHE[key] = (build_m1(trivial), build_m2(trivial))
    return _CACHE[key]


def _is_trivial(inp):
    z = lambda k: not np.any(inp[k])
    one = lambda k: bool(np.all(inp[k] == 1.0))
    return (z("b_qvk") and z("b_kv") and z("b_q") and z("b_ff2")
            and one("ln1_g") and z("ln1_b") and one("ln2_g") and z("ln2_b")
            and one("ln3_g") and z("ln3_b"))


def kernel(**inputs):
    inp = {k: np.ascontiguousarray(np.asarray(v)) for k, v in inputs.items()}
    trivial = _is_trivial(inp)
    m1, m2 = _modules(trivial)

    shared_m1 = {"W_qvk": inp["W_qvk"], "W_kv": inp["W_kv"]}
    if not trivial:
        shared_m1.update({"b_qvk": inp["b_qvk"], "b_kv": inp["b_kv"]})
    in1 = []
    for c in range(N_CORES):
        b, q = c // 4, c % 4
        in1.append(dict(
            x=inp["x"][b, q * T:(q + 1) * T],
            mem=inp["memory"][b, q * T:(q + 1) * T],
            **shared_m1,
        ))
    r1 = bass_utils.run_bass_kernel_spmd(m1, in1, core_ids=list(range(N_CORES)))
    res1 = r1.results

    # host-side exchange: causal prefix (exclusive scan) + ctx all-reduce
    s_tot = [res1[c]["s_o"].sum(axis=0) for c in range(N_CORES)]
    p_core = []
    ctx_full = []
    for c in range(N_CORES):
        grp = GROUPS[c // 4]
        pc = np.zeros((H * E, E + 1), np.float32)
        for cc in grp:
            if cc < c:
                pc = pc + s_tot[cc]
        p_core.append(pc)
        ctx_full.append(sum(res1[cc]["ctx_o"] for cc in grp))

    shared_m2 = {"W_q": inp["W_q"], "W_ff1": inp["W_ff1"],
                 "W_ff2": inp["W_ff2"], "b_ff1": inp["b_ff1"]}
    if not trivial:
        shared_m2.update({k: inp[k] for k in (
            "b_q", "b_ff2", "ln1_g", "ln1_b", "ln2_g", "ln2_b",
            "ln3_g", "ln3_b")})
    in2 = []
    for c in range(N_CORES):
        b, q = c // 4, c % 4
        in2.append(dict(
            x=inp["x"][b, q * T:(q + 1) * T],
            qsT_i=res1[c]["qsT_o"], poi_i=res1[c]["poi_o"],
            p_core=p_core[c], s_i=res1[c]["s_o"], ctx=ctx_full[c],
            **shared_m2,
        ))
    r2 = bass_utils.run_bass_kernel_spmd(m2, in2, core_ids=list(range(N_CORES)))
    global LAST_EXEC_NS
    LAST_EXEC_NS = [r1.exec_time_ns, r2.exec_time_ns]

    out = np.zeros((B, N, DIM), np.float32)
    for c in range(N_CORES):
        b, q = c // 4, c % 4
        out[b, q * T:(q + 1) * T] = r2.results[c]["out"]
    return out

